# revision 7
# baseline (speedup 1.0000x reference)
"""NonLocalBlock (B=4, C=256, H=W=64) Trainium2 Bass kernel, v2.

Sharding: 8 cores = 4 batch elements x 2 query-row shards of 2048 rows.
Each core receives its batch element's x rotated along N so that its
query rows are columns [0, 2048) -- pure SPMD.

Per-core pipeline (engine-balanced, software-pipelined):
  A) projections from fp16 x (host-cast):
       theta[d, nq], phi[d, m] -> cast fp8e4 and repartitioned to
       [64, 2, *] (DoubleRow layout) via SBUF->SBUF DMA;
       gT[m%128, (mc, d)] fp16 (plain value projection, g-bias dropped:
       training-mode BN cancels channel constants);
       c[m] = SCALE * btheta^T phi_m  (32 tiny matmuls; the theta-bias
       enters the softmax only as this per-key logit shift).
  B) attention, q-block-paired (jp in {0,1} covers 1024 q-cols):
       per m-chunk mc:
         S^T halves = phi8_mc x theta8 (fp8 DoubleRow matmuls, PE)
         pT = exp(SCALE*S^T + c_mc)   (one ScalarE instr per 1024 cols,
                                       bias operand = c column)
         y_ps += gT_mc.T @ pT ; r_ps += ones.T @ pT   (fp16 PE, PSUM)
       then ysb = y (fp16), rho = 1/r (DVE reciprocal_approx_fast),
       out_norm = (wo.T @ ysb) * rho with s1/s2 accumulated via DVE stt.
  C) BN stats AllReduce [128, 4] across 8 cores; out = x + a*out_norm + b
     with a = gamma*rstd, b = beta - a*mean.

float is fp16 end-to-end except: PSUM f32 accumulation, BN stats f32,
and the fp8e4 logit operands (error budget ~1e-2 vs 2e-2 tolerance).
"""

import math

import numpy as np

import concourse.bass as bass
import concourse.mybir as mybir
import concourse.tile as tile
from concourse import bacc
from concourse.bass_utils import run_bass_kernel_spmd

# Problem constants (hardcoded per contract).
B, C, HGT, WID = 4, 256, 64, 64
N = HGT * WID            # 4096 spatial positions
D = C // 2               # 128 inner channels
P = 128                  # SBUF partitions
HP = P // 2              # 64 = half partitions (DoubleRow layout)
NCORES = 8
SPLIT = NCORES // B      # query shards per batch element
NQ = N // SPLIT          # 2048 query rows per core
CB = C // P              # 2 channel chunks
MCH = N // P             # 32 key chunks
NBLK = 512               # max moving free dim
JP = 2                   # query pair-blocks of 1024
JW = NQ // JP            # 1024 query cols per jp
EPS = 1e-5
SCALE = 1.0 / math.sqrt(D)
BTH_S = 16.0             # host-side btheta prescale (fp8 range)
NSAMP = float(B * N)     # BN sample count per channel

F32 = mybir.dt.float32
F16 = mybir.dt.float16
F8 = mybir.dt.float8e4

AF = mybir.ActivationFunctionType
ALU = mybir.AluOpType
PM = mybir.MatmulPerfMode

_CACHED_NC = None


def _compile_with_joint_act_tables(nc):
    """Force Exp and Ln onto the joint `natural_log_exp_and_others` table
    set so no ACT_TABLE_LOAD (1283ns) fires mid-kernel."""
    real = bacc.get_activation_tables

    def patched(arch):
        t = dict(real(arch))
        for k in ("exp_and_others", "natural_log"):
            if k in t:
                t[k] = type(t[k])()
        return t

    bacc.get_activation_tables = patched
    try:
        nc.compile()
    finally:
        bacc.get_activation_tables = real


def _build_nc():
    nc = bacc.Bacc("TRN2", target_bir_lowering=False, debug=False,
                   num_devices=NCORES)

    # x, fp16, rotated per core: [c%128, cb*N + n]
    x_d = nc.dram_tensor("x16", [P, CB * N], F16, kind="ExternalInput")
    # fp16 weights: wq | wk | wv | wo (each [128, cb*128+col])
    wp_d = nc.dram_tensor("wpack", [P, 4 * C], F16, kind="ExternalInput")
    # btheta in DoubleRow layout, prescaled by BTH_S: [64, 2]
    bt_d = nc.dram_tensor("bth8", [HP, 2], F8, kind="ExternalInput")
    # f32 consts: gamma (2 cols) | beta (2 cols) | bphi (1 col)
    cp_d = nc.dram_tensor("cpack", [P, 2 * CB + 1], F32, kind="ExternalInput")
    out_d = nc.dram_tensor("out", [C, NQ], F32, kind="ExternalOutput")

    with tile.TileContext(nc) as tc:
        with (
            tc.tile_pool(name="consts", bufs=1) as consts,
            tc.tile_pool(name="bigs", bufs=1) as bigs,
            tc.tile_pool(name="ptp", bufs=3) as ptp,
            tc.tile_pool(name="work", bufs=2) as work,
            tc.tile_pool(name="ps_s", bufs=2, space="PSUM") as ps_s,
            tc.tile_pool(name="ps_y", bufs=1, space="PSUM") as ps_y,
            tc.tile_pool(name="ps_r", bufs=1, space="PSUM") as ps_r,
            tc.tile_pool(name="dram", bufs=1, space="DRAM") as dram,
        ):
            # ---- constant / weight loads ----
            wpack = consts.tile([P, 4 * C], F16)
            bth8 = consts.tile([HP, 2, 1], F8)
            cpack = consts.tile([P, 2 * CB + 1], F32)
            nc.gpsimd.dma_start(wpack[:], wp_d[:])
            nc.gpsimd.dma_start(bth8[:], bt_d[:])
            nc.gpsimd.dma_start(cpack[:], cp_d[:])
            wq = wpack[:, 0 * C:1 * C]
            wk = wpack[:, 1 * C:2 * C]
            wv = wpack[:, 2 * C:3 * C]
            wo = wpack[:, 3 * C:4 * C]
            gam = cpack[:, 0:CB]
            bet = cpack[:, CB:2 * CB]
            bphi = cpack[:, 2 * CB:2 * CB + 1]
            ones = consts.tile([P, P], F16)
            nc.vector.memset(ones[:], 1.0)

            # ---- x load (fp16), 8 chunks so phase A can start early ----
            x16 = bigs.tile([P, CB * N], F16, tag="x16")
            QCH = N // 1024
            for k in range(QCH):
                for cb in range(CB):
                    sl = slice(cb * N + k * 1024, cb * N + (k + 1) * 1024)
                    nc.gpsimd.dma_start(x16[:, sl], x_d[:, sl])

            # ---- phase A ----
            # staging tiles ([128, *] fp8) later repartitioned to [64, 2, *]
            th_st = bigs.tile([P, NQ], F8, tag="th_st")
            ph_st = bigs.tile([P, N], F8, tag="ph_st")
            th8 = bigs.tile([HP, 2, NQ], F8, tag="th8")
            ph8 = bigs.tile([HP, 2, N], F8, tag="ph8")
            gT = bigs.tile([P, N], F16, tag="gT")  # [m%128, mc*128 + d]
            c16 = consts.tile([P, MCH], F32)

            # theta: q in [0, 2048) -- x quarters 0..1
            for k in range(NQ // 1024):
                pt = ps_s.tile([P, 1024], F32, tag="ps")
                for h in range(2):
                    hsl = slice(h * NBLK, (h + 1) * NBLK)
                    xsl = slice(k * 1024 + h * NBLK, k * 1024 + (h + 1) * NBLK)
                    for cb in range(CB):
                        nc.tensor.matmul(
                            pt[:, hsl], wq[:, cb * P:(cb + 1) * P],
                            x16[:, cb * N + xsl.start:cb * N + xsl.stop],
                            start=(cb == 0), stop=(cb == CB - 1))
                # no theta bias: folded into the exp bias (c vector)
                nc.scalar.activation(th_st[:, k * 1024:(k + 1) * 1024],
                                     pt[:], AF.Copy)
            for i in range(2):
                nc.sync.dma_start(th8[:, i, :], th_st[i * HP:(i + 1) * HP, :])

            # phi: all N, per 1024-quarter; then repartition + c matmuls
            for k in range(QCH):
                pt = ps_s.tile([P, 1024], F32, tag="ps")
                for h in range(2):
                    hsl = slice(h * NBLK, (h + 1) * NBLK)
                    for cb in range(CB):
                        xo = cb * N + k * 1024 + h * NBLK
                        nc.tensor.matmul(
                            pt[:, hsl], wk[:, cb * P:(cb + 1) * P],
                            x16[:, xo:xo + NBLK],
                            start=(cb == 0), stop=(cb == CB - 1))
                nc.scalar.activation(ph_st[:, k * 1024:(k + 1) * 1024],
                                     pt[:], AF.Identity, bias=bphi)
            for i in range(2):
                nc.sync.dma_start(ph8[:, i, :], ph_st[i * HP:(i + 1) * HP, :])

            # c[m] = btheta^T phi_m (DoubleRow tiny matmuls, 1 col each)
            c_ps = ps_y.tile([P, 1024], F32, tag="ps_y")
            for mc in range(MCH):
                nc.tensor.matmul(
                    c_ps[:, mc:mc + 1], ph8[:, :, mc * P:(mc + 1) * P],
                    bth8[:], perf_mode=PM.DoubleRow,
                    start=True, stop=True)
            nc.vector.tensor_scalar_mul(c16[:], c_ps[:, 0:MCH],
                                        SCALE / BTH_S)

            # g -> gT chunks [m, d]; 4 chunks per PSUM tile; bias dropped
            for k in range(MCH // 8):  # 4 groups of 8 chunks = [128,1024]
                gp = ps_s.tile([P, 1024], F32, tag="ps")
                for q in range(8):
                    mc = k * 8 + q
                    for cb in range(CB):
                        nc.tensor.matmul(
                            gp[:, q * P:(q + 1) * P],
                            x16[:, cb * N + mc * P:cb * N + (mc + 1) * P],
                            wv[:, cb * P:(cb + 1) * P],
                            start=(cb == 0), stop=(cb == CB - 1))
                nc.vector.tensor_copy(gT[:, k * 1024:(k + 1) * 1024], gp[:])

            # ---- phase B: attention (software-pipelined flat loop) ----
            outs = bigs.tile([P, CB * NQ], F32, tag="outs")  # [c%128, cb*NQ+q]
            s1 = consts.tile([P, 2 * JP * CB], F32)  # cols: jp*CB+cb | +4: sq
            NIT = JP * MCH  # 64

            y_ps = r_ps = None
            pts = [None] * NIT

            def post_block(jp):
                # runs after the last y/r matmul of pair-block jp
                ysb = work.tile([P, JW], F16, tag="ysb")
                nc.vector.tensor_copy(ysb[:], y_ps[:])
                rho = work.tile([P, JW], F32, tag="rho")
                nc.vector.reciprocal_approx_fast(rho[:], r_ps[:])
                for cb in range(CB):
                    o_ps = ps_s.tile([P, JW], F32, tag="ps")
                    for h in range(2):
                        hsl = slice(h * NBLK, (h + 1) * NBLK)
                        nc.tensor.matmul(o_ps[:, hsl],
                                         wo[:, cb * P:(cb + 1) * P],
                                         ysb[:, hsl], start=True, stop=True)
                    osl = slice(cb * NQ + jp * JW, cb * NQ + (jp + 1) * JW)
                    col = jp * CB + cb
                    nc.vector.scalar_tensor_tensor(
                        out=outs[:, osl], in0=o_ps[:], scalar=1.0,
                        in1=rho[:], op0=ALU.mult, op1=ALU.mult,
                        accum_out=s1[:, col:col + 1])
                    sq = work.tile([P, JW], F32, tag="sq")
                    nc.vector.scalar_tensor_tensor(
                        out=sq[:], in0=outs[:, osl], scalar=1.0,
                        in1=outs[:, osl], op0=ALU.mult, op1=ALU.mult,
                        accum_out=s1[:, 4 + col:5 + col])

            for it in range(NIT + 2):
                jp, mc = divmod(it, MCH)
                if it < NIT:
                    # S^T pair + exp for iteration `it`
                    s_ps = ps_s.tile([P, 2 * NBLK], F32, tag="ps")
                    for h in range(2):
                        qo = jp * JW + h * NBLK
                        nc.tensor.matmul(
                            s_ps[:, h * NBLK:(h + 1) * NBLK],
                            ph8[:, :, mc * P:(mc + 1) * P],
                            th8[:, :, qo:qo + NBLK],
                            perf_mode=PM.DoubleRow, start=True, stop=True)
                    pt = ptp.tile([P, 2 * NBLK], F16, tag="pT")
                    nc.scalar.activation(pt[:], s_ps[:], AF.Exp, scale=SCALE,
                                         bias=c16[:, mc:mc + 1])
                    pts[it] = pt
                # y/r quad for iteration `it-1` (one-iteration lag)
                pit = it - 1
                if 0 <= pit < NIT:
                    pjp, pmc = divmod(pit, MCH)
                    if pmc == 0:
                        y_ps = ps_y.tile([P, JW], F32, tag="ps_y")
                        r_ps = ps_r.tile([P, JW], F32, tag="ps_r")
                    pt = pts[pit]
                    pts[pit] = None
                    msl = slice(pmc * P, (pmc + 1) * P)
                    for h in range(2):
                        hsl = slice(h * NBLK, (h + 1) * NBLK)
                        nc.tensor.matmul(y_ps[:, hsl], gT[:, msl], pt[:, hsl],
                                         start=(pmc == 0), stop=(pmc == MCH - 1))
                    for h in range(2):
                        hsl = slice(h * NBLK, (h + 1) * NBLK)
                        nc.tensor.matmul(r_ps[:, hsl], ones[:], pt[:, hsl],
                                         start=(pmc == 0), stop=(pmc == MCH - 1))
                    if pmc == MCH - 1:
                        post_block(pjp)

            # ---- phase C: BN stats allreduce + apply + residual ----
            gin = consts.tile([P, 2 * CB], F32)
            nc.vector.tensor_add(gin[:, 0:CB], s1[:, 0:CB], s1[:, CB:2 * CB])
            nc.vector.tensor_add(gin[:, CB:2 * CB], s1[:, 4:4 + CB],
                                 s1[:, 4 + CB:4 + 2 * CB])

            cc_in = dram.tile([P, 2 * CB], F32)
            cc_out = dram.tile([P, 2 * CB], F32)
            nc.sync.dma_start(cc_in[:], gin[:])
            nc.gpsimd.collective_compute(
                "AllReduce", ALU.add,
                replica_groups=[list(range(NCORES))],
                ins=[cc_in[:].opt()], outs=[cc_out[:].opt()])
            gstats = consts.tile([P, 2 * CB], F32)
            nc.sync.dma_start(gstats[:], cc_out[:])

            mean = consts.tile([P, CB], F32)
            var = consts.tile([P, CB], F32)
            tmp = consts.tile([P, CB], F32)
            rstd = consts.tile([P, CB], F32)
            a_sc = consts.tile([P, CB], F32)
            b_sc = consts.tile([P, CB], F32)
            nc.vector.tensor_scalar_mul(mean[:], gstats[:, 0:CB], 1.0 / NSAMP)
            nc.vector.tensor_mul(tmp[:], mean[:], mean[:])
            nc.vector.scalar_tensor_tensor(
                out=var[:], in0=gstats[:, CB:2 * CB], scalar=1.0 / NSAMP,
                in1=tmp[:], op0=ALU.mult, op1=ALU.subtract)
            eps_t = consts.tile([P, 1], F32)
            nc.vector.memset(eps_t[:], EPS)
            nc.scalar.activation(tmp[:], var[:], AF.Ln, bias=eps_t[:])
            nc.scalar.activation(rstd[:], tmp[:], AF.Exp, scale=-0.5)
            nc.vector.tensor_mul(a_sc[:], gam[:], rstd[:])
            nc.vector.tensor_mul(tmp[:], a_sc[:], mean[:])
            nc.vector.tensor_sub(b_sc[:], bet[:], tmp[:])

            for cb in range(CB):
                xb = work.tile([P, NQ], F32, tag="xb")
                nc.scalar.activation(xb[:], x16[:, cb * N:cb * N + NQ],
                                     AF.Identity, bias=b_sc[:, cb:cb + 1])
                for h in range(2):
                    hsl = slice(h * JW, (h + 1) * JW)
                    f = work.tile([P, JW], F32, tag="f")
                    nc.vector.scalar_tensor_tensor(
                        out=f[:], in0=outs[:, cb * NQ + h * JW:
                                           cb * NQ + (h + 1) * JW],
                        scalar=a_sc[:, cb:cb + 1], in1=xb[:, hsl],
                        op0=ALU.mult, op1=ALU.add)
                    nc.sync.dma_start(out_d[cb * P:(cb + 1) * P, hsl], f[:])

    _compile_with_joint_act_tables(nc)
    return nc


def _get_nc():
    global _CACHED_NC
    if _CACHED_NC is None:
        _CACHED_NC = _build_nc()
    return _CACHED_NC


def _in_maps(inputs):
    f8 = mybir.dt.np(F8)
    x = np.ascontiguousarray(
        np.asarray(inputs["x"], np.float32)).reshape(B, C, N)
    tw = np.asarray(inputs["theta_w"], np.float32)
    pw = np.asarray(inputs["phi_w"], np.float32)
    gw = np.asarray(inputs["g_w"], np.float32)
    ow = np.asarray(inputs["out_w"], np.float32)

    def pack_ct(w):  # [Dout, C] -> [128, CB*Dout] chunk-major transposed
        wt = np.ascontiguousarray(w.T)            # [C, Dout]
        return np.concatenate([wt[cb * P:(cb + 1) * P, :] for cb in range(CB)],
                              axis=1)

    owt = np.ascontiguousarray(ow.T)              # [D, C], cols cb*128+cc
    wpack = np.concatenate(
        [pack_ct(tw), pack_ct(pw), pack_ct(gw), owt],
        axis=1).astype(np.float16)                # [128, 4*256]

    bth = np.asarray(inputs["theta_b"], np.float32) * BTH_S
    bth8 = np.stack([bth[0:HP], bth[HP:P]], axis=1).astype(f8)  # [64, 2]

    gam = np.asarray(inputs["gamma"], np.float32).reshape(CB, P).T
    bet = np.asarray(inputs["beta"], np.float32).reshape(CB, P).T
    bphi = np.asarray(inputs["phi_b"], np.float32).reshape(P, 1)
    cpack = np.ascontiguousarray(np.concatenate([gam, bet, bphi], axis=1))

    maps = []
    for core in range(NCORES):
        b, h = divmod(core, SPLIT)
        n0 = h * NQ
        xr = x[b] if n0 == 0 else np.concatenate(
            [x[b][:, n0:], x[b][:, :n0]], axis=1)
        x16 = np.ascontiguousarray(
            xr.reshape(CB, P, N).transpose(1, 0, 2).reshape(P, CB * N)
        ).astype(np.float16)
        maps.append({"x16": x16, "wpack": wpack, "bth8": bth8,
                     "cpack": cpack})
    return maps


def _run(inputs, trace=False, **kw):
    nc = _get_nc()
    maps = _in_maps(inputs)
    r = run_bass_kernel_spmd(nc, maps, list(range(NCORES)), trace=trace, **kw)
    out = np.empty((B, C, N), np.float32)
    for core in range(NCORES):
        b, h = divmod(core, SPLIT)
        out[b][:, h * NQ:(h + 1) * NQ] = r.results[core]["out"]
    return out.reshape(B, C, HGT, WID), r


def kernel(**inputs):
    out, _ = _run(inputs, trace=False)
    return out


# revision 10
# speedup vs baseline: 1.0770x; 1.0770x over previous
"""NonLocalBlock (B=4, C=256, H=W=64) Trainium2 Bass kernel, v3.

Sharding: 8 cores = 4 batch elements x 2 query-row shards of 2048 rows.
Each core receives its batch element's x rotated along N so that its
query rows are columns [0, 2048) -- pure SPMD.

Per-core pipeline (engine-balanced, software-pipelined):
  A) projections from fp16 x (host-cast):
       theta[d, nq] fp16 (NO bias -- theta-bias enters the softmax only
       as a per-key logit shift, folded into the exp bias operand);
       phi[d, m] fp16 (with bias);
       gT[m%128, (mc, d)] fp16 (g-bias dropped: training-mode BN
       cancels channel constants; so is the out-conv bias);
       c[m] = SCALE * btheta^T phi_m  (32 single-column matmuls).
  B) attention, q-block-paired (jp in {0,1} covers 1024 q-cols), flat
     loop over 64 (jp, mc) iterations with a 2-iteration lag between
     the S^T/exp front end and the y/r back end so the PE never idles
     (idle resets the Tensor-engine p-state ramp: 2.4GHz -> 1.2GHz):
       S^T halves = phi_mc x theta (fp16, PE)
       pT = exp(SCALE*S^T + c_mc)  (one ScalarE instr per 1024 cols,
                                    bias operand = c column)
       y_ps += gT_mc.T @ pT ; r_ps += ones.T @ pT  (fp16 PE, PSUM)
     per jp: ysb=y (fp16), rho=1/r (DVE reciprocal_approx_fast),
     out_norm = (wo.T @ ysb) * rho, s1/s2 accumulated via DVE stt.
  C) BN stats AllReduce [128, 4] across 8 cores; out = x + a*out_norm
     + b with a = gamma*rstd, b = beta - a*mean.
"""

import math

import numpy as np

import concourse.bass as bass
import concourse.mybir as mybir
import concourse.tile as tile
from concourse import bacc
from concourse.bass_utils import run_bass_kernel_spmd

# Problem constants (hardcoded per contract).
B, C, HGT, WID = 4, 256, 64, 64
N = HGT * WID            # 4096 spatial positions
D = C // 2               # 128 inner channels
P = 128                  # SBUF partitions
NCORES = 8
SPLIT = NCORES // B      # query shards per batch element
NQ = N // SPLIT          # 2048 query rows per core
CB = C // P              # 2 channel chunks
MCH = N // P             # 32 key chunks
NBLK = 512               # max moving free dim
JP = 2                   # query pair-blocks of 1024
JW = NQ // JP            # 1024 query cols per jp
LAG = 2                  # exp -> y/r software pipeline depth
EPS = 1e-5
SCALE = 1.0 / math.sqrt(D)
NSAMP = float(B * N)     # BN sample count per channel

F32 = mybir.dt.float32
F16 = mybir.dt.float16

AF = mybir.ActivationFunctionType
ALU = mybir.AluOpType

_CACHED_NC = None


def _compile_with_joint_act_tables(nc):
    """Force Exp and Ln onto the joint `natural_log_exp_and_others` table
    set so no ACT_TABLE_LOAD (1283ns) fires mid-kernel."""
    real = bacc.get_activation_tables

    def patched(arch):
        t = dict(real(arch))
        for k in ("exp_and_others", "natural_log"):
            if k in t:
                t[k] = type(t[k])()
        return t

    bacc.get_activation_tables = patched
    try:
        nc.compile()
    finally:
        bacc.get_activation_tables = real


def _build_nc():
    nc = bacc.Bacc("TRN2", target_bir_lowering=False, debug=False,
                   num_devices=NCORES)

    # x, fp16, rotated per core: [c%128, cb*N + n]
    x_d = nc.dram_tensor("x16", [P, CB * N], F16, kind="ExternalInput")
    # fp16 weights: wq | wk | wv | wo (each [128, cb*128+col]) | btheta
    wp_d = nc.dram_tensor("wpack", [P, 4 * C + 1], F16, kind="ExternalInput")
    # f32 consts: gamma (2 cols) | beta (2 cols) | bphi (1 col)
    cp_d = nc.dram_tensor("cpack", [P, 2 * CB + 1], F32, kind="ExternalInput")
    out_d = nc.dram_tensor("out", [C, NQ], F32, kind="ExternalOutput")

    with tile.TileContext(nc) as tc:
        with (
            tc.tile_pool(name="consts", bufs=1) as consts,
            tc.tile_pool(name="bigs", bufs=1) as bigs,
            tc.tile_pool(name="ptp", bufs=2 + LAG) as ptp,
            tc.tile_pool(name="work", bufs=2) as work,
            tc.tile_pool(name="ps_s", bufs=2, space="PSUM") as ps_s,
            tc.tile_pool(name="ps_y", bufs=1, space="PSUM") as ps_y,
            tc.tile_pool(name="ps_r", bufs=1, space="PSUM") as ps_r,
            tc.tile_pool(name="dram", bufs=1, space="DRAM") as dram,
        ):
            # ---- constant / weight loads ----
            wpack = consts.tile([P, 4 * C + 1], F16)
            cpack = consts.tile([P, 2 * CB + 1], F32)
            nc.sync.dma_start(wpack[:], wp_d[:])
            nc.sync.dma_start(cpack[:], cp_d[:])
            wq = wpack[:, 0 * C:1 * C]
            wk = wpack[:, 1 * C:2 * C]
            wv = wpack[:, 2 * C:3 * C]
            wo = wpack[:, 3 * C:4 * C]
            bth = wpack[:, 4 * C:4 * C + 1]
            gam = cpack[:, 0:CB]
            bet = cpack[:, CB:2 * CB]
            bphi = cpack[:, 2 * CB:2 * CB + 1]
            ones = consts.tile([P, P], F16)
            nc.vector.memset(ones[:], 1.0)

            # ---- x load (fp16), 8 chunks split across two DMA queues ----
            x16 = bigs.tile([P, CB * N], F16, tag="x16")
            QCH = N // 1024
            for k in range(QCH):
                for cb in range(CB):
                    sl = slice(cb * N + k * 1024, cb * N + (k + 1) * 1024)
                    eng = nc.gpsimd if cb == 0 else nc.scalar
                    eng.dma_start(x16[:, sl], x_d[:, sl])

            # ---- phase A ----
            th16 = bigs.tile([P, NQ], F16, tag="th16")
            ph16 = bigs.tile([P, N], F16, tag="ph16")
            gT = bigs.tile([P, N], F16, tag="gT")  # [m%128, mc*128 + d]
            c16 = consts.tile([P, MCH], F32)

            # theta: q in [0, 2048) -- x quarters 0..1; no bias
            for k in range(NQ // 1024):
                pt = ps_s.tile([P, 1024], F32, tag="ps")
                for h in range(2):
                    hsl = slice(h * NBLK, (h + 1) * NBLK)
                    for cb in range(CB):
                        xo = cb * N + k * 1024 + h * NBLK
                        nc.tensor.matmul(
                            pt[:, hsl], wq[:, cb * P:(cb + 1) * P],
                            x16[:, xo:xo + NBLK],
                            start=(cb == 0), stop=(cb == CB - 1))
                nc.scalar.activation(th16[:, k * 1024:(k + 1) * 1024],
                                     pt[:], AF.Copy)

            # phi: all N, with bias
            for k in range(QCH):
                pt = ps_s.tile([P, 1024], F32, tag="ps")
                for h in range(2):
                    hsl = slice(h * NBLK, (h + 1) * NBLK)
                    for cb in range(CB):
                        xo = cb * N + k * 1024 + h * NBLK
                        nc.tensor.matmul(
                            pt[:, hsl], wk[:, cb * P:(cb + 1) * P],
                            x16[:, xo:xo + NBLK],
                            start=(cb == 0), stop=(cb == CB - 1))
                nc.scalar.activation(ph16[:, k * 1024:(k + 1) * 1024],
                                     pt[:], AF.Identity, bias=bphi)

            # c[m] = btheta^T phi_m (single-column matmuls into one bank)
            c_ps = ps_y.tile([P, JW], F32, tag="ps_y")
            for mc in range(MCH):
                nc.tensor.matmul(
                    c_ps[:, mc:mc + 1], ph16[:, mc * P:(mc + 1) * P],
                    bth[:], start=True, stop=True)
            nc.vector.tensor_scalar_mul(c16[:], c_ps[:, 0:MCH], SCALE)

            # g -> gT chunks [m, d]; 8 chunks per PSUM tile; bias dropped
            for k in range(MCH // 8):
                gp = ps_s.tile([P, 1024], F32, tag="ps")
                for q in range(8):
                    mc = k * 8 + q
                    for cb in range(CB):
                        nc.tensor.matmul(
                            gp[:, q * P:(q + 1) * P],
                            x16[:, cb * N + mc * P:cb * N + (mc + 1) * P],
                            wv[:, cb * P:(cb + 1) * P],
                            start=(cb == 0), stop=(cb == CB - 1))
                nc.vector.tensor_copy(gT[:, k * 1024:(k + 1) * 1024], gp[:])

            # ---- phase B: attention (lag-LAG software-pipelined) ----
            outs = bigs.tile([P, CB * NQ], F32, tag="outs")  # [c%128, cb*NQ+q]
            s1 = consts.tile([P, 2 * JP * CB], F32)  # jp*CB+cb | +4: squares
            NIT = JP * MCH  # 64

            y_ps = r_ps = None
            pts = [None] * NIT
            posts = [None] * (NIT + LAG + 2)

            def post_front(jp):
                # ysb / rho on DVE, issued right after the last y/r matmul
                ysb = work.tile([P, JW], F16, tag="ysb")
                nc.vector.tensor_copy(ysb[:], y_ps[:])
                rho = work.tile([P, JW], F32, tag="rho")
                nc.vector.reciprocal_approx_fast(rho[:], r_ps[:])

                def back():
                    for cb in range(CB):
                        o_ps = ps_s.tile([P, JW], F32, tag="ps")
                        for h in range(2):
                            hsl = slice(h * NBLK, (h + 1) * NBLK)
                            nc.tensor.matmul(o_ps[:, hsl],
                                             wo[:, cb * P:(cb + 1) * P],
                                             ysb[:, hsl],
                                             start=True, stop=True)
                        osl = slice(cb * NQ + jp * JW, cb * NQ + (jp + 1) * JW)
                        col = jp * CB + cb
                        nc.vector.scalar_tensor_tensor(
                            out=outs[:, osl], in0=o_ps[:], scalar=1.0,
                            in1=rho[:], op0=ALU.mult, op1=ALU.mult,
                            accum_out=s1[:, col:col + 1])
                        sq = work.tile([P, JW], F32, tag="sq")
                        nc.vector.scalar_tensor_tensor(
                            out=sq[:], in0=outs[:, osl], scalar=1.0,
                            in1=outs[:, osl], op0=ALU.mult, op1=ALU.mult,
                            accum_out=s1[:, 4 + col:5 + col])
                return back

            for it in range(NIT + LAG + 1):
                jp, mc = divmod(it, MCH)
                if it < NIT:
                    # S^T pair + exp for iteration `it`
                    s_ps = ps_s.tile([P, 2 * NBLK], F32, tag="ps")
                    for h in range(2):
                        qo = jp * JW + h * NBLK
                        nc.tensor.matmul(
                            s_ps[:, h * NBLK:(h + 1) * NBLK],
                            ph16[:, mc * P:(mc + 1) * P],
                            th16[:, qo:qo + NBLK], start=True, stop=True)
                    pt = ptp.tile([P, 2 * NBLK], F16, tag="pT")
                    nc.scalar.activation(pt[:], s_ps[:], AF.Exp, scale=SCALE,
                                         bias=c16[:, mc:mc + 1])
                    pts[it] = pt
                # out-conv + stats for a finished jp, one iter after its
                # ysb/rho were issued (keeps the PE stream stall-free)
                if posts[it] is not None:
                    posts[it]()
                    posts[it] = None
                # y/r quad for iteration `it-LAG`
                pit = it - LAG
                if 0 <= pit < NIT:
                    pjp, pmc = divmod(pit, MCH)
                    if pmc == 0:
                        y_ps = ps_y.tile([P, JW], F32, tag="ps_y")
                        r_ps = ps_r.tile([P, JW], F32, tag="ps_r")
                    pt = pts[pit]
                    pts[pit] = None
                    msl = slice(pmc * P, (pmc + 1) * P)
                    for h in range(2):
                        hsl = slice(h * NBLK, (h + 1) * NBLK)
                        nc.tensor.matmul(y_ps[:, hsl], gT[:, msl], pt[:, hsl],
                                         start=(pmc == 0), stop=(pmc == MCH - 1))
                    for h in range(2):
                        hsl = slice(h * NBLK, (h + 1) * NBLK)
                        nc.tensor.matmul(r_ps[:, hsl], ones[:], pt[:, hsl],
                                         start=(pmc == 0), stop=(pmc == MCH - 1))
                    if pmc == MCH - 1:
                        posts[it + 1] = post_front(pjp)

            # ---- phase C: BN stats all-reduce + apply + residual ----
            gin = consts.tile([P, 2 * CB], F32)
            nc.vector.tensor_add(gin[:, 0:CB], s1[:, 0:CB], s1[:, CB:2 * CB])
            nc.vector.tensor_add(gin[:, CB:2 * CB], s1[:, 4:4 + CB],
                                 s1[:, 4 + CB:4 + 2 * CB])

            # Direct all-to-all instead of collective_compute (whose ring +
            # launch costs ~35us for 2KB): every core broadcasts its [128,4]
            # partial stats to each peer's slot (XOR-relative Δtpb=b, so the
            # SPMD program is identical on all cores), then tree-reduces the
            # 8 slots locally. Remote writes are invisible to the Tile
            # dependency tracker, so an explicit semaphore wait (+2 per
            # incoming write, 7 peers -> 14) gates the reduce.
            slots = consts.tile([P, NCORES * 2 * CB], F32)
            rsem = nc.alloc_semaphore("stats_rsem")
            lsem = nc.alloc_semaphore("stats_lsem")
            nc.vector.tensor_copy(slots[:, 0:2 * CB], gin[:])
            for b in range(1, NCORES):
                rd = [None] * NCORES
                rd[b] = (0, b)
                nc.gpsimd.remote_dma_broadcast(
                    slots[:, b * 2 * CB:(b + 1) * 2 * CB], gin[:],
                    rsem, lsem, rdests=rd)
            nc.gpsimd.trigger_dma(count=None)
            nc.vector.wait_ge(rsem, 2 * (NCORES - 1))
            red1 = consts.tile([P, NCORES * CB], F32)
            red2 = consts.tile([P, NCORES], F32)
            gstats = consts.tile([P, 2 * CB], F32)
            nc.vector.tensor_add(red1[:], slots[:, 0:16], slots[:, 16:32])
            nc.vector.tensor_add(red2[:], red1[:, 0:8], red1[:, 8:16])
            nc.vector.tensor_add(gstats[:], red2[:, 0:4], red2[:, 4:8])

            mean = consts.tile([P, CB], F32)
            var = consts.tile([P, CB], F32)
            tmp = consts.tile([P, CB], F32)
            rstd = consts.tile([P, CB], F32)
            a_sc = consts.tile([P, CB], F32)
            b_sc = consts.tile([P, CB], F32)
            nc.vector.tensor_scalar_mul(mean[:], gstats[:, 0:CB], 1.0 / NSAMP)
            nc.vector.tensor_mul(tmp[:], mean[:], mean[:])
            nc.vector.scalar_tensor_tensor(
                out=var[:], in0=gstats[:, CB:2 * CB], scalar=1.0 / NSAMP,
                in1=tmp[:], op0=ALU.mult, op1=ALU.subtract)
            eps_t = consts.tile([P, 1], F32)
            nc.vector.memset(eps_t[:], EPS)
            nc.scalar.activation(tmp[:], var[:], AF.Ln, bias=eps_t[:])
            nc.scalar.activation(rstd[:], tmp[:], AF.Exp, scale=-0.5)
            nc.vector.tensor_mul(a_sc[:], gam[:], rstd[:])
            nc.vector.tensor_mul(tmp[:], a_sc[:], mean[:])
            nc.vector.tensor_sub(b_sc[:], bet[:], tmp[:])

            for cb in range(CB):
                xb = work.tile([P, NQ], F32, tag="xb")
                nc.scalar.activation(xb[:], x16[:, cb * N:cb * N + NQ],
                                     AF.Identity, bias=b_sc[:, cb:cb + 1])
                for h in range(2):
                    hsl = slice(h * JW, (h + 1) * JW)
                    f = work.tile([P, JW], F32, tag="f")
                    nc.vector.scalar_tensor_tensor(
                        out=f[:], in0=outs[:, cb * NQ + h * JW:
                                           cb * NQ + (h + 1) * JW],
                        scalar=a_sc[:, cb:cb + 1], in1=xb[:, hsl],
                        op0=ALU.mult, op1=ALU.add)
                    nc.sync.dma_start(out_d[cb * P:(cb + 1) * P, hsl], f[:])

    _compile_with_joint_act_tables(nc)
    return nc


def _get_nc():
    global _CACHED_NC
    if _CACHED_NC is None:
        _CACHED_NC = _build_nc()
    return _CACHED_NC


def _in_maps(inputs):
    x = np.ascontiguousarray(
        np.asarray(inputs["x"], np.float32)).reshape(B, C, N)
    tw = np.asarray(inputs["theta_w"], np.float32)
    pw = np.asarray(inputs["phi_w"], np.float32)
    gw = np.asarray(inputs["g_w"], np.float32)
    ow = np.asarray(inputs["out_w"], np.float32)

    def pack_ct(w):  # [Dout, C] -> [128, CB*Dout] chunk-major transposed
        wt = np.ascontiguousarray(w.T)            # [C, Dout]
        return np.concatenate([wt[cb * P:(cb + 1) * P, :] for cb in range(CB)],
                              axis=1)

    owt = np.ascontiguousarray(ow.T)              # [D, C], cols cb*128+cc
    bth = np.asarray(inputs["theta_b"], np.float32).reshape(P, 1)
    wpack = np.concatenate(
        [pack_ct(tw), pack_ct(pw), pack_ct(gw), owt, bth],
        axis=1).astype(np.float16)                # [128, 4*256+1]

    gam = np.asarray(inputs["gamma"], np.float32).reshape(CB, P).T
    bet = np.asarray(inputs["beta"], np.float32).reshape(CB, P).T
    bphi = np.asarray(inputs["phi_b"], np.float32).reshape(P, 1)
    cpack = np.ascontiguousarray(np.concatenate([gam, bet, bphi], axis=1))

    maps = []
    for core in range(NCORES):
        b, h = divmod(core, SPLIT)
        n0 = h * NQ
        xr = x[b] if n0 == 0 else np.concatenate(
            [x[b][:, n0:], x[b][:, :n0]], axis=1)
        x16 = np.ascontiguousarray(
            xr.reshape(CB, P, N).transpose(1, 0, 2).reshape(P, CB * N)
        ).astype(np.float16)
        maps.append({"x16": x16, "wpack": wpack, "cpack": cpack})
    return maps


def _run(inputs, trace=False, **kw):
    nc = _get_nc()
    maps = _in_maps(inputs)
    r = run_bass_kernel_spmd(nc, maps, list(range(NCORES)), trace=trace, **kw)
    out = np.empty((B, C, N), np.float32)
    for core in range(NCORES):
        b, h = divmod(core, SPLIT)
        out[b][:, h * NQ:(h + 1) * NQ] = r.results[core]["out"]
    return out.reshape(B, C, HGT, WID), r


def kernel(**inputs):
    out, _ = _run(inputs, trace=False)
    return out
